# revision 11
# baseline (speedup 1.0000x reference)
"""Trainium2 Bass kernel for nn_Decoder (Tacotron-style decoder).

Data-parallel over batch B=16 across 8 NeuronCores (B=2 per core), bf16
matmuls (validated ~7e-3 mel rel err). Per core: prenet/attention GEMMs via
matmul_tile_kernel; 3 GRU scans (blender + 2 residual) with feature-major
layout and weights-stationary 128x128 matmul tiles. Biases enter through an
extra ones-row appended to each contraction (padded to a 128 multiple).
"""

import contextlib
import numpy as np
import ml_dtypes

import concourse.bass as bass
import concourse.bacc as bacc
import concourse.mybir as mybir
import concourse.tile as tile
from concourse import bass_utils
from concourse.bass import ds
from concourse.kernels.tile_matmul import matmul_tile_kernel

F32 = mybir.dt.float32
BF16 = mybir.dt.bfloat16
AF = mybir.ActivationFunctionType
ALU = mybir.AluOpType

B, S, T, I, C, M, A, H = 16, 256, 1024, 512, 1024, 80, 128, 256
L = 2
G3 = 3 * C
NCORES = 8
BC = B // NCORES          # 2 batch elems per core
KT = C // 128             # 8 k tiles
MT = G3 // 128            # 24 m tiles
NB = KT * BC              # 16 cols per gate block
KB = I + H + 128          # 896: blender contraction padded (768 + ones + zeros)
KG = C + 128              # 1152: gru/proj contraction padded
KP2 = H + 128             # 384: prenet2 contraction padded
LOG_EPS = float(np.log(1e-5))

_compiled = {}


def bf(x):
    return np.ascontiguousarray(np.asarray(x, np.float32).astype(ml_dtypes.bfloat16))


def build_nc(TT=T):
    RR = BC * TT
    nc = bacc.Bacc(None, target_bir_lowering=False)
    dp = nc.declare_dram_parameter

    shiftedT = dp("shiftedT", [M + 1, RR], F32, isOutput=False)
    inputs_rows = dp("inputs_rows", [BC, S, I], F32, isOutput=False)
    inputsT = dp("inputsT", [BC, I, S], F32, isOutput=False)
    maskbias = dp("maskbias", [BC, TT, S], F32, isOutput=False)
    pw1T = dp("pw1T", [M + 1, H], BF16, isOutput=False)
    pw2T = dp("pw2T", [KP2, H], BF16, isOutput=False)
    aqT = dp("aqT", [H, A], BF16, isOutput=False)
    akT = dp("akT", [I, A], BF16, isOutput=False)
    wihT0 = dp("wihT0", [KB, G3], BF16, isOutput=False)
    wihT1 = dp("wihT1", [KG, G3], BF16, isOutput=False)
    wihT2 = dp("wihT2", [KG, G3], BF16, isOutput=False)
    wihT = [wihT0, wihT1, wihT2]
    whhP = [dp(f"whhP{l}", [128, KT * MT * 128], BF16, isOutput=False) for l in range(3)]
    bhnE = [dp(f"bhnE{l}", [128, NB], F32, isOutput=False) for l in range(3)]
    projT = dp("projT", [KG, 128], BF16, isOutput=False)
    pad_rows = dp("pad_rows", [128, RR], BF16, isOutput=False)  # row0=ones, rest 0

    alphaT = dp("alphaT", [BC, TT, S], F32, isOutput=True)
    melT = dp("melT", [128, RR], F32, isOutput=True)

    h1b = nc.dram_tensor("h1b", [KP2, RR], BF16)
    concatT = nc.dram_tensor("concatT", [KB, RR], BF16)
    qm = nc.dram_tensor("qm", [A, RR], BF16)
    kTb = nc.dram_tensor("kTb", [BC, A, S], BF16)
    eT = nc.dram_tensor("eT", [BC, TT, S], F32)
    alpha_bf = nc.dram_tensor("alpha_bf", [BC, TT, S], BF16)
    xw = [nc.dram_tensor(f"xw{l}", [G3, RR], BF16) for l in range(3)]
    xk = [nc.dram_tensor(f"xk{l}", [KG, RR], BF16) for l in range(3)]

    with contextlib.ExitStack() as ctx, tile.TileContext(nc) as tc:
        def mm(kxm, kxn, mxn, **kw):
            kw.setdefault("matmul_dtype", BF16)
            matmul_tile_kernel(tc, kxm, kxn, mxn, **kw)

        # bias/pad rows into padded contraction buffers
        nc.sync.dma_start(h1b[H:KP2, :], pad_rows[:, :])
        nc.sync.dma_start(concatT[I + H:KB, :], pad_rows[:, :])
        for l in range(3):
            nc.sync.dma_start(xk[l][C:KG, :], pad_rows[:, :])

        # G1 prenet1: h1 [H, RR] = pw1T.T @ shiftedT   (K=81, single partial tile)
        mm(pw1T[:, :], shiftedT[:, :], h1b[0:H, :], use_relu=True)
        # G2 prenet2 -> preproc in concatT rows [I:I+H]
        mm(pw2T[:, :], h1b[:, :], concatT[I:I + H, :], use_relu=True)
        # G3 q = aqT.T @ preproc (aq pre-scaled by 1/sqrt(A))
        mm(aqT[:, :], concatT[I:I + H, :], qm[:, :])
        # G4 per-batch k_bT [A, S]
        for b in range(BC):
            mm(akT[:, :], inputsT[b], kTb[b])
        # G5 per-batch eT_b [TT, S] = q_b.T @ k_bT + maskbias
        for b in range(BC):
            mm(qm[:, b * TT:(b + 1) * TT], kTb[b], eT[b], accumulate_ap=maskbias[b])

        # softmax along S; writes alphaT
        ntile = max(1, TT // 128)
        P = min(128, TT)
        with tc.tile_pool(name="sm", bufs=3) as smp, \
             tc.tile_pool(name="smst", bufs=4) as stp:
            for b in range(BC):
                for i in range(ntile):
                    et = smp.tile([P, S], F32, tag="et")
                    nc.sync.dma_start(et[:, :], eT[b][i * P:(i + 1) * P, :])
                    mx = stp.tile([P, 1], F32, tag="mx")
                    nc.vector.tensor_reduce(mx[:, :], et[:, :],
                                            axis=mybir.AxisListType.X, op=ALU.max)
                    nmx = stp.tile([P, 1], F32, tag="nmx")
                    nc.vector.tensor_scalar_mul(nmx[:, :], mx[:, :], -1.0)
                    ex = smp.tile([P, S], F32, tag="ex")
                    sm = stp.tile([P, 1], F32, tag="sm")
                    nc.scalar.activation(ex[:, :], et[:, :], AF.Exp, bias=nmx[:, 0:1],
                                         accum_out=sm[:, 0:1])
                    rs = stp.tile([P, 1], F32, tag="rs")
                    nc.vector.reciprocal(rs[:, :], sm[:, :])
                    al = smp.tile([P, S], F32, tag="al")
                    nc.vector.tensor_scalar_mul(al[:, :], ex[:, :], rs[:, 0:1])
                    nc.sync.dma_start(alphaT[b][i * P:(i + 1) * P, :], al[:, :])
                    albf = smp.tile([P, S], BF16, tag="albf")
                    nc.vector.tensor_copy(albf[:, :], al[:, :])
                    nc.sync.dma_start(alpha_bf[b][i * P:(i + 1) * P, :], albf[:, :])

        # G6 alignedT_b [I, TT] = inputs_rows_b.T @ alpha_b -> concatT[0:I, b-block]
        for b in range(BC):
            mm(inputs_rows[b], alpha_bf[b], concatT[0:I, b * TT:(b + 1) * TT],
               transpose_kxn=True)

        # G7 blender input projection
        mm(wihT0[:, :], concatT[:, :], xw[0][:, :])

        # ---- GRU scans ----
        for l in range(3):
            with tc.tile_pool(name=f"wpool{l}", bufs=1) as wp, \
                 tc.tile_pool(name=f"cpool{l}", bufs=1) as cp, \
                 tc.tile_pool(name=f"hpool{l}", bufs=1) as hp, \
                 tc.tile_pool(name=f"spool{l}", bufs=3) as sp, \
                 tc.tile_pool(name=f"gpool{l}", bufs=2) as gp, \
                 tc.tile_pool(name=f"pspool{l}", bufs=2, space="PSUM") as pp:
                wsb = wp.tile([128, KT * MT * 128], BF16, tag="wsb")
                nc.sync.dma_start(wsb[:, :], whhP[l][:, :])
                bhn = cp.tile([128, NB], F32, tag="bhn")
                nc.sync.dma_start(bhn[:, :], bhnE[l][:, :])
                h32 = hp.tile([128, NB], F32, tag="h32")
                hbf = hp.tile([128, NB], BF16, tag="hbf")
                nc.vector.memset(h32[:, :], 0.0)
                nc.vector.memset(hbf[:, :], 0.0)

                xw3 = xw[l][:, :].rearrange("(m p) (b t) -> p m b t", p=128, b=BC)
                xin3 = (xk[l - 1][0:C, :].rearrange("(k p) (b t) -> p k b t", p=128, b=BC)
                        if l > 0 else None)
                xo3 = xk[l][0:C, :].rearrange("(k p) (b t) -> p k b t", p=128, b=BC)

                with tc.For_i(0, TT, 1) as t:
                    slab = sp.tile([128, MT * BC], BF16, tag="slab")
                    for b_ in range(BC):
                        nc.sync.dma_start(
                            slab[:, :].rearrange("p (m b) -> p m b", b=BC)[:, :, b_],
                            xw3[:, :, b_, ds(t, 1)])
                    if l > 0:
                        xin = sp.tile([128, NB], BF16, tag="xin")
                        for b_ in range(BC):
                            nc.sync.dma_start(
                                xin[:, :].rearrange("p (k b) -> p k b", b=BC)[:, :, b_],
                                xin3[:, :, b_, ds(t, 1)])
                    ps = pp.tile([128, MT * BC], F32, tag="ps")
                    for m in range(MT):
                        for k in range(KT):
                            nc.tensor.matmul(
                                ps[:, m * BC:(m + 1) * BC],
                                wsb[:, (k * MT + m) * 128:(k * MT + m) * 128 + 128],
                                hbf[:, k * BC:(k + 1) * BC],
                                start=(k == 0), stop=(k == KT - 1))
                    ar = gp.tile([128, NB], F32, tag="ar")
                    az = gp.tile([128, NB], F32, tag="az")
                    nc.vector.tensor_add(ar[:, :], ps[:, 0:NB], slab[:, 0:NB])
                    nc.vector.tensor_add(az[:, :], ps[:, NB:2 * NB], slab[:, NB:2 * NB])
                    r = gp.tile([128, NB], F32, tag="r")
                    z = gp.tile([128, NB], F32, tag="z")
                    nc.scalar.activation(r[:, :], ar[:, :], AF.Sigmoid)
                    nc.scalar.activation(z[:, :], az[:, :], AF.Sigmoid)
                    hn = gp.tile([128, NB], F32, tag="hn")
                    nc.vector.tensor_add(hn[:, :], ps[:, 2 * NB:3 * NB], bhn[:, :])
                    t1 = gp.tile([128, NB], F32, tag="t1")
                    nc.vector.tensor_mul(t1[:, :], r[:, :], hn[:, :])
                    t2 = gp.tile([128, NB], F32, tag="t2")
                    nc.vector.tensor_add(t2[:, :], t1[:, :], slab[:, 2 * NB:3 * NB])
                    n = gp.tile([128, NB], F32, tag="n")
                    nc.scalar.activation(n[:, :], t2[:, :], AF.Tanh)
                    d = gp.tile([128, NB], F32, tag="d")
                    nc.vector.tensor_sub(d[:, :], h32[:, :], n[:, :])
                    zd = gp.tile([128, NB], F32, tag="zd")
                    nc.vector.tensor_mul(zd[:, :], z[:, :], d[:, :])
                    nc.vector.tensor_add(h32[:, :], n[:, :], zd[:, :])
                    nc.vector.tensor_copy(hbf[:, :], h32[:, :])
                    xo = gp.tile([128, NB], BF16, tag="xo")
                    if l > 0:
                        nc.vector.tensor_add(xo[:, :], h32[:, :], xin[:, :])
                    else:
                        nc.vector.tensor_copy(xo[:, :], h32[:, :])
                    for b_ in range(BC):
                        nc.sync.dma_start(
                            xo3[:, :, b_, ds(t, 1)],
                            xo[:, :].rearrange("p (k b) -> p k b", b=BC)[:, :, b_])

            if l < 2:
                mm(wihT[l + 1][:, :], xk[l][:, :], xw[l + 1][:, :])

        mm(projT[:, :], xk[2][:, :], melT[:, :])

    nc.compile()
    return nc


def prep_inputs(inputs, TT=T):
    x = {k: np.asarray(v, np.float32) for k, v in inputs.items()}
    gt, mask, inp = x["gt"], x["mask"], x["inputs"]
    shifted = np.pad(gt, ((0, 0), (1, 0), (0, 0)), constant_values=LOG_EPS)[:, :TT]
    RR = BC * TT

    pw1T = bf(np.concatenate([x["pw1"].T, x["pb1"][None, :]], 0))
    pw2T_np = np.zeros((KP2, H), np.float32)
    pw2T_np[0:H] = x["pw2"].T
    pw2T_np[H] = x["pb2"]
    aqT = bf(x["aq"].T / np.sqrt(np.float32(A)))
    akT = bf(x["ak"].T)
    projT_np = np.zeros((KG, 128), np.float32)
    projT_np[0:C, 0:M] = x["projw"].T
    projT_np[C, 0:M] = x["projb"]

    wih_all = [x["bwih"], x["gwih"][0], x["gwih"][1]]
    whh_all = [x["bwhh"], x["gwhh"][0], x["gwhh"][1]]
    bih_all = [x["bbih"], x["gbih"][0], x["gbih"][1]]
    bhh_all = [x["bbhh"], x["gbhh"][0], x["gbhh"][1]]
    wihT_l, whhP_l, bhnE_l = [], [], []
    for l in range(3):
        wih, whh, bih, bhh = wih_all[l], whh_all[l], bih_all[l], bhh_all[l]
        din = wih.shape[1]
        kpad = KB if l == 0 else KG
        wt = np.zeros((kpad, G3), np.float32)
        wt[0:din] = wih.T
        bias_row = bih.copy()
        bias_row[0:2 * C] += bhh[0:2 * C]
        wt[din] = bias_row
        wihT_l.append(bf(wt))
        w4 = whh.T.reshape(KT, 128, MT, 128)                # [k, p, m, q]
        whhP_l.append(bf(np.transpose(w4, (1, 0, 2, 3)).reshape(128, KT * MT * 128)))
        bhn = bhh[2 * C:3 * C].reshape(KT, 128).T            # [p, k]
        bhnE_l.append(np.ascontiguousarray(
            np.repeat(bhn[:, :, None], BC, axis=2).reshape(128, NB).astype(np.float32)))

    pad = np.zeros((128, RR), np.float32)
    pad[0] = 1.0
    pad = bf(pad)

    maps = []
    for c in range(NCORES):
        sl = slice(c * BC, (c + 1) * BC)
        shT = np.transpose(shifted[sl], (2, 0, 1)).reshape(M, RR)
        shT = np.concatenate([shT, np.ones((1, RR), np.float32)], 0)
        mb = np.where(mask[sl] > 0, 0.0, -1e9).astype(np.float32)
        maps.append({
            "shiftedT": np.ascontiguousarray(shT),
            "inputs_rows": np.ascontiguousarray(inp[sl]),
            "inputsT": np.ascontiguousarray(np.transpose(inp[sl], (0, 2, 1))),
            "maskbias": np.ascontiguousarray(
                np.broadcast_to(mb[:, None, :], (BC, TT, S)).astype(np.float32)),
            "pw1T": pw1T, "pw2T": bf(pw2T_np), "aqT": aqT, "akT": akT,
            "projT": bf(projT_np),
            "wihT0": wihT_l[0], "wihT1": wihT_l[1], "wihT2": wihT_l[2],
            **{f"whhP{l}": whhP_l[l] for l in range(3)},
            **{f"bhnE{l}": bhnE_l[l] for l in range(3)},
            "pad_rows": pad,
        })
    return maps


def kernel(**inputs):
    if "nc" not in _compiled:
        _compiled["nc"] = build_nc(T)
    nc = _compiled["nc"]
    in_maps = prep_inputs(inputs, T)
    res = bass_utils.run_bass_kernel_spmd(nc, in_maps, core_ids=list(range(NCORES)))
    mel = np.zeros((B, T, M), np.float32)
    alpha = np.zeros((B, T, S), np.float32)
    for c in range(NCORES):
        melT = res.results[c]["melT"]
        alphaT = res.results[c]["alphaT"]
        for b in range(BC):
            mel[c * BC + b] = melT[0:M, b * T:(b + 1) * T].T
            alpha[c * BC + b] = alphaT[b]
    return mel, alpha
